# revision 5
# baseline (speedup 1.0000x reference)
"""ChebGraphConv (K=3) Trainium2 kernel.

y = x@(W0-W2) - (A@x)@W1 + 2*A@((A@x)@W2) + bias

computed per (b,t) slice as:
  P0 = X@W02 ; P1 = X@W1 ; P2' = X@(2*W2)
  Q' = A@P2' ; M = P1 - Q' ; S = A@M ; y = P0 - S (+bias)

The spmms are fp8e5m2 DoubleRow matmuls. With `--enable-ldw-opt=false`
(hardcoded in this toolchain) every matmul reloads its stationary, so each
DR matmul costs ~LDWEIGHTS(256 cols) ~= 209 ns regardless of free size; the
kernel is weight-load bound and the only lever is the matmul COUNT. All 12
slices of a core are therefore pushed through each A-pass at FD=512 (PSUM
bank cap): the 1536 moving columns of work (P2' 768 | M 768) are split into
three 512-wide pass tiles

  R0 = P2'[slices 0-7]
  R1 = [P2' slices 8-11 | M slices 0-3]
  R2 = [M slices 4-11]

and three A-passes p0/p1/p2 run chain(k): sum_mt A^T[k,mt] @ Rp[mt] with one
FD=512 matmul per stationary -> 3*128 = 384 DR matmuls instead of the
4*128 = 512 a 6-slice-group structure needs. Each pass reads a tile written
only by earlier phases (proj -> R0,R1; p0 -> R1,R2; p1 -> R2), so coarse
tile-dependency tracking introduces no false PE serialization.

PSUM evacuations: M = P1 - psum (fp8 out), y = P0 - psum (f32), both DVE
tensor_tensor reading PSUM; proj copies (P2' fp8 strided into R-tiles, P1/P0
bf16) split between DVE and Act. Data parallel over B: core b handles x[b];
y is written [N, T, C] per core and transposed on host.
"""

import numpy as np
import ml_dtypes

import concourse.bacc as bacc
import concourse.mybir as mybir
import concourse.tile as tile
from concourse import bass_utils

BF16 = ml_dtypes.bfloat16
FP8E5 = ml_dtypes.float8_e5m2

B, T, N, C = 8, 12, 2048, 64
NB = N // 128          # 16 node blocks
NMT = NB // 2          # 8 DoubleRow contraction slabs (256 nodes each)
FD = 512               # PSUM-bank-capped matmul free size (8 slices x 64)

_NC_CACHE = {}


def _build_nc(repeat=None, with_bias=False):
    """repeat=None: single-shot kernel (graded path). repeat=R: wraps the
    whole body in a hardware For loop running it R times (benchmarking)."""
    key = ("nc", repeat, with_bias)
    if key in _NC_CACHE:
        return _NC_CACHE[key]
    f32 = mybir.dt.float32
    bf16 = mybir.dt.bfloat16
    fp8 = mybir.dt.float8e5

    nc = bacc.Bacc("TRN2", target_bir_lowering=False, debug=False,
                   enable_asserts=False, num_devices=8)

    at_d = nc.dram_tensor("at8", [NB, 128, NMT, 256], fp8,
                          kind="ExternalInput")
    xs_d = nc.dram_tensor("xs", [T // 2, 128, N], bf16, kind="ExternalInput")
    wa_d = nc.dram_tensor("wa", [128, 3 * C], bf16, kind="ExternalInput")
    bias_d = nc.dram_tensor("biasb", [128, C], f32, kind="ExternalInput")
    y_d = nc.dram_tensor("y", [N, T, C], f32, kind="ExternalOutput")

    with tile.TileContext(nc) as tc:
        with (
            tc.tile_pool(name="const", bufs=1) as constp,
            tc.tile_pool(name="atp", bufs=2) as atp,
            tc.tile_pool(name="xsp", bufs=2) as xsp,
            tc.tile_pool(name="rp", bufs=1) as rp,
            tc.tile_pool(name="pp10", bufs=1) as pp10,
            tc.tile_pool(name="ystage", bufs=4) as ystage,
            tc.tile_pool(name="pps", bufs=2, space="PSUM") as pps,
            tc.tile_pool(name="sps", bufs=4, space="PSUM") as sps,
        ):
            def emit_body():
                _emit(nc, constp, atp, xsp, rp, pp10, ystage, pps, sps,
                      at_d, xs_d, wa_d, bias_d, y_d, with_bias)

            if repeat is None:
                emit_body()
            else:
                with tc.For_i(0, repeat, 1):
                    emit_body()

    nc.compile()
    _NC_CACHE[key] = nc
    return nc


def _emit(nc, constp, atp, xsp, rp, pp10, ystage, pps, sps,
          at_d, xs_d, wa_d, bias_d, y_d, with_bias):
    f32 = mybir.dt.float32
    bf16 = mybir.dt.bfloat16
    fp8 = mybir.dt.float8e5

    wa_t = constp.tile([128, 3 * C], bf16, tag="wa")
    bias_t = constp.tile([128, C], f32, tag="bias")

    xs_t = [xsp.tile([128, N], bf16, tag=f"xs{p}", name=f"xs{p}")
            for p in range(T // 2)]
    at_t = [atp.tile([128, NMT, 256], fp8, tag=f"at{k}", name=f"at{k}")
            for k in range(NB)]
    # pass tiles: [128, slab, plane, 512]; R0=P2'[0-8), R1=[P2'[8-12)|M[0-4)],
    # R2=M[4-12). plane = node-block parity within the 256-node slab.
    r_t = [rp.tile([128, NMT, 2, FD], fp8, tag=f"r{p}", name=f"r{p}")
           for p in range(3)]
    p1_t = pp10.tile([128, NB, T * C], bf16, tag="p1", name="p1")
    p0_t = pp10.tile([128, NB, T * C], bf16, tag="p0", name="p0")

    nc.sync.dma_start(xs_t[0][:], xs_d[0, :, :])
    nc.sync.dma_start(wa_t[:], wa_d[:, :])
    for p in range(1, T // 2):
        nc.sync.dma_start(xs_t[p][:], xs_d[p, :, :])
    for k in range(NB):
        nc.sync.dma_start(at_t[k][:], at_d[k, :, :, :])
    nc.sync.dma_start(bias_t[:], bias_d[:, :])

    def xstat(s, k):
        """Stationary [64, 128] for slice s, node block k."""
        return xs_t[s // 2][(s % 2) * C:(s % 2 + 1) * C, k * 128:(k + 1) * 128]

    def proj_step(s, q):
        """Projection for slice s, node blocks 4q..4q+3.
        pp columns per block j (at 256-col spacing): [P1|P2'|P0|pad]."""
        h = (s % 2) * C
        pp = pps.tile([128, 1024], f32, tag="pp", name="pp")
        for j in range(4):
            k = 4 * q + j
            nc.tensor.matmul(pp[:, j * 256:j * 256 + 192],
                             xstat(s, k),
                             wa_t[h:h + C, :], start=True, stop=True)
        pv = pp.rearrange("p (j2 i pl c) -> p j2 i pl c", j2=2, i=2, pl=4, c=C)
        # P2' -> pass tile (fp8): slices 0-7 -> R0, 8-11 -> R1 cols 0:256
        rt, s0 = (r_t[0], s) if s < 8 else (r_t[1], s - 8)
        rv = rt.rearrange("p m i (sl c) -> p m i sl c", sl=FD // C, c=C)
        nc.vector.tensor_copy(rv[:, 2 * q:2 * q + 2, :, s0, :],
                              pv[:, :, :, 1, :])
        # P1 / P0 -> bf16 staging (Act engine)
        cs = slice(s * C, (s + 1) * C)
        nc.scalar.copy(p1_t[:, 4 * q:4 * q + 4, cs],
                       pp.rearrange("p (j pl c) -> p j pl c",
                                    j=4, pl=4, c=C)[:, :, 0, :])
        nc.scalar.copy(p0_t[:, 4 * q:4 * q + 4, cs],
                       pp.rearrange("p (j pl c) -> p j pl c",
                                    j=4, pl=4, c=C)[:, :, 2, :])

    def interleave_emit(interleave, k, total_k):
        if not interleave:
            return
        nchunk = len(interleave)
        c0 = k * nchunk // total_k
        c1 = (k + 1) * nchunk // total_k
        for thunk in interleave[c0:c1]:
            thunk()

    def chain(p, k):
        """sp = A[kblk,:] @ Rp  (full 2048 contraction, FD=512)."""
        sp = sps.tile([128, FD], f32, tag="sp", name="sp")
        for mt in range(NMT):
            nc.tensor.matmul(sp[:], at_t[k][:, mt, :],
                             r_t[p][:, mt, :, :],
                             start=(mt == 0), stop=(mt == NMT - 1),
                             perf_mode=mybir.MatmulPerfMode.DoubleRowSwInterleave)
        return sp

    def m_dst(k, s0, ns):
        """R-tile destination for M slices [s0, s0+ns): slices 0-3 -> R1
        cols 256:512, slices 4-11 -> R2."""
        if s0 < 4:
            return r_t[1][:, k // 2, k % 2, 256 + s0 * C:256 + (s0 + ns) * C]
        return r_t[2][:, k // 2, k % 2, (s0 - 4) * C:(s0 - 4 + ns) * C]

    def emit_y(k, s0, ns, src):
        yt = ystage.tile([128, FD], f32, tag="y", name="yt")
        nc.vector.tensor_sub(yt[:, :ns * C], p0_t[:, k, s0 * C:(s0 + ns) * C],
                             src)
        if with_bias:
            for i in range(ns):
                ysl = yt[:, i * C:(i + 1) * C]
                nc.vector.tensor_tensor(ysl, ysl, bias_t[:],
                                        op=mybir.AluOpType.add)
        nc.sync.dma_start(y_d[k * 128:(k + 1) * 128, s0:s0 + ns, :],
                          yt[:, :ns * C])

    # proj slices 0-7 (fill R0 + P1/P0); slices 8-11 interleave into pass 0
    for s in range(8):
        for q in range(NB // 4):
            proj_step(s, q)
    proj_rest = [(lambda ss=s, qq=q: proj_step(ss, qq))
                 for s in range(8, T) for q in range(NB // 4)]

    # pass 0: spmm2 slices 0-7 -> M[0-4) in R1, M[4-8) in R2
    for k in range(NB):
        interleave_emit(proj_rest, k, NB)
        sp = chain(0, k)
        nc.vector.tensor_tensor(m_dst(k, 0, 4), p1_t[:, k, 0:4 * C],
                                sp[:, 0:4 * C], op=mybir.AluOpType.subtract)
        nc.vector.tensor_tensor(m_dst(k, 4, 4), p1_t[:, k, 4 * C:8 * C],
                                sp[:, 4 * C:8 * C], op=mybir.AluOpType.subtract)

    # pass 1: spmm2 slices 8-11 -> M[8-12) in R2 ; spmm3 slices 0-3 -> y
    for k in range(NB):
        sp = chain(1, k)
        nc.vector.tensor_tensor(m_dst(k, 8, 4), p1_t[:, k, 8 * C:12 * C],
                                sp[:, 0:4 * C], op=mybir.AluOpType.subtract)
        emit_y(k, 0, 4, sp[:, 4 * C:8 * C])

    # pass 2: spmm3 slices 4-11 -> y
    for k in range(NB):
        sp = chain(2, k)
        emit_y(k, 4, 8, sp[:])


def _prep_inputs(x, A_norm, weight, bias):
    """Host-side shard + layout prep. Returns per-core input maps."""
    x = np.asarray(x, dtype=np.float32)
    A_norm = np.asarray(A_norm, dtype=np.float32)
    weight = np.asarray(weight, dtype=np.float32)
    bias = np.asarray(bias, dtype=np.float32)

    # per-k-block DoubleRowSwInterleave A^T pack: for each (kb, mt) the
    # 256-wide stationary holds [A127,B127,A126,B126,...,A0,B0] per
    # partition kp, where A/B = planes i=0/1 and column n' runs reversed:
    # at8[kb, kp, mt, 2*(127-n')+i] = A[kb*128+n', mt*256 + i*128 + kp]
    A2 = A_norm.reshape(NB, 128, NMT, 2, 128)        # [kb, n', mt, i, kp]
    at8 = A2.transpose(0, 4, 2, 1, 3)[:, :, :, ::-1, :]  # [kb, kp, mt, n'r, i]
    at8_host = np.ascontiguousarray(at8.reshape(NB, 128, NMT, 256)).astype(FP8E5)

    W0, W1, W2 = weight[0], weight[1], weight[2]
    wa_host = np.zeros((128, 3 * C), dtype=BF16)
    for h in (0, C):
        wa_host[h:h + C, 0:C] = W1.astype(BF16)
        wa_host[h:h + C, C:2 * C] = (2.0 * W2).astype(BF16)
        wa_host[h:h + C, 2 * C:3 * C] = (W0 - W2).astype(BF16)

    bias_host = np.ascontiguousarray(np.broadcast_to(bias, (128, C)),
                                     dtype=np.float32)

    in_maps = []
    for b in range(B):
        xt = x[b].transpose(0, 2, 1)                 # [T, C, N]
        xt = xt.reshape(T // 2, 128, N)              # pair slices on partitions
        in_maps.append({
            "at8": at8_host,
            "xs": np.ascontiguousarray(xt).astype(BF16),
            "wa": wa_host,
            "biasb": bias_host,
        })
    return in_maps


def kernel(x, A_norm, weight, bias):
    with_bias = bool(np.any(np.asarray(bias)))
    nc = _build_nc(with_bias=with_bias)
    in_maps = _prep_inputs(x, A_norm, weight, bias)
    last_err = None
    for attempt in range(3):
        try:
            res = bass_utils.run_bass_kernel_spmd(nc, in_maps,
                                                  core_ids=list(range(8)))
            break
        except Exception as e:  # transient NRT_EXEC_UNIT_UNRECOVERABLE etc.
            last_err = e
            import time
            time.sleep(2.0 * (attempt + 1))
    else:
        raise last_err
    # per-core y is [N, T, C]; full output is [B, T, N, C]
    out = np.stack([res.results[b]["y"].transpose(1, 0, 2) for b in range(B)],
                   axis=0)
    return np.ascontiguousarray(out, dtype=np.float32)
